# revision 16
# baseline (speedup 1.0000x reference)
"""Trainium2 Bass kernel for the DMP (dynamic movement primitives) rollout.

Math: the reference rollout is, per dimension d, a linear 2-state recurrence
    s_t = A s_{t-1} + B u_t,   s = [y; dy],  s_0 = [y0; 0]
with constant A (2x2), B = [dt^2; dt], and forcing
    u_t[d] = ALPHA_Y*BETA_Y*g[d] + sum_j phi_t[j] * weights[d,j]*(g[d]-y0[d])
where phi_t[j] = WEIGHT_SCALE * psi_t[j] * x_t / sum(psi_t) depends only on
constants (x_t = decay^t is input-independent).  By superposition the whole
trajectory factors through an input-independent basis:
    y_t[d], dy_t[d] = sum_m BB[t, comp, m] * coeff[m, d]       (m = 0..26)
with channels m = 0..24 the 25 basis-forced responses (coeff w[:,j]*(g-y0)),
m = 25 the homogeneous response (coeff y0), m = 26 the step response with
ALPHA_Y*BETA_Y folded in (coeff g).

Per core (time rows sharded across 8 cores, no cross-core comm):
  - coeff (27 x 1024) is computed on device from the raw y0/g/weights inputs
    (DVE stream transposes + a DMA partition-broadcast of g-y0),
  - the y/dy output blocks are a [2502, 27] @ [27, 1024] tensor-engine matmul,
  - the y0-replica block is written by broadcast DMA from an SBUF staging
    tile (no HBM reads, no compute).
"""

import numpy as np

DIM = 1024
NB = 25
ALPHA_X = 1.0
DT = 0.001
MAX_TIME = 10.0
TAU = 1.0
ALPHA_Y = 25.0
BETA_Y = 6.25
WEIGHT_SCALE = 1000.0
T = int(MAX_TIME / DT) + 1        # 10001

NCORES = 8
RPC = 1251                        # t-rows per core; 8*1251 = 10008 >= T
R2 = RPC * 2                      # 2502 matmul rows per core (y and dy)
R2PAD = 2560                      # 20 tiles of 128
NMT = R2PAD // 128                # 20
M = 2 + NB                        # 27 basis channels

F32R = True                       # tensor-engine fast-fp32 mode (4x matmul)

_cache = {}


def _basis_slices():
    """Per-core transposed basis slices: list of [M, R2PAD] float32 arrays."""
    if "bbT" in _cache:
        return _cache["bbT"]
    f32 = np.float32
    # phi replicated in fp32 with the reference op order
    c = np.exp(-ALPHA_X * np.linspace(0.0, MAX_TIME, NB, dtype=f32)).astype(f32)
    h = (NB / c).astype(f32)
    decay = f32(1.0 - ALPHA_X * TAU * DT)
    x = f32(1.0)
    phi = np.zeros((T - 1, NB), dtype=np.float64)
    for t in range(T - 1):
        x = f32(x * decay)
        d = (x - c).astype(f32)
        arg = (h * (d * d).astype(f32)).astype(f32)
        psi = np.exp(-arg).astype(f32)
        s = f32(psi.sum(dtype=f32))
        phi[t] = (psi.astype(np.float64) * float(x) * WEIGHT_SCALE) / float(s)

    dt = TAU * DT
    a, b = ALPHA_Y, BETA_Y
    A = np.array([[1 - dt * dt * a * b, dt * (1 - dt * a)],
                  [-dt * a * b, 1 - dt * a]], dtype=np.float64)
    B = np.array([dt * dt, dt], dtype=np.float64)
    # internal channel order: 0 homogeneous (E), 1 step (S), 2.. forced (C)
    Z = np.zeros((2, M), dtype=np.float64)
    Z[0, 0] = 1.0
    # output channel order (must match device rhs rows):
    #   m = 0..24 -> C_j (coeff w.T*(g-y0)); m = 25 -> E (coeff y0);
    #   m = 26 -> ALPHA_Y*BETA_Y*S (coeff g, scale folded into the basis)
    BB = np.zeros((T, 2, M), dtype=np.float64)
    BB[0, 0, 25] = 1.0                 # y_0 = y0 (dy_0 row stays zero)
    u = np.zeros(M)
    u[1] = 1.0
    for t in range(1, T):
        u[2:] = phi[t - 1]
        Z = A @ Z + np.outer(B, u)
        for comp in (0, 1):
            BB[t, comp, :25] = Z[comp, 2:]
            BB[t, comp, 25] = Z[comp, 0]
            BB[t, comp, 26] = (a * b) * Z[comp, 1]

    flat = np.zeros((NCORES * R2, M), dtype=f32)
    flat[: T * 2] = BB.reshape(T * 2, M).astype(f32)
    slices = []
    for i in range(NCORES):
        bbT = np.zeros((M, R2PAD), dtype=f32)
        bbT[:, :R2] = flat[i * R2:(i + 1) * R2].T
        slices.append(np.ascontiguousarray(bbT))
    _cache["bbT"] = slices
    return slices


def _program():
    """Build (once) the Bass/Tile program shared by all 8 cores."""
    if "nc" in _cache:
        return _cache["nc"]
    import concourse.mybir as mybir
    import concourse.tile as tile
    from concourse import bacc

    f32 = mybir.dt.float32
    mmdt = mybir.dt.float32r if F32R else f32
    nc = bacc.Bacc("TRN2", target_bir_lowering=False, debug=False,
                   enable_asserts=False, num_devices=NCORES)
    bbT_h = nc.dram_tensor("bbT", [M, R2PAD], f32, kind="ExternalInput")
    y0_h = nc.dram_tensor("y0", [1, DIM], f32, kind="ExternalInput")
    g_h = nc.dram_tensor("g", [1, DIM], f32, kind="ExternalInput")
    w_h = nc.dram_tensor("w", [8, 128, NB], f32, kind="ExternalInput")
    out_h = nc.dram_tensor("out", [RPC, 3, DIM], f32, kind="ExternalOutput")

    with tile.TileContext(nc) as tc:
        with (
            tc.tile_pool(name="const", bufs=1) as const,
            tc.tile_pool(name="dram", bufs=1, space="DRAM") as dram,
            tc.tile_pool(name="psMM", bufs=4, space="PSUM") as psMM,
            tc.tile_pool(name="outp", bufs=3) as outp,
        ):
            outv = out_h.ap()

            bb_s = const.tile([M, R2PAD], f32)
            nc.sync.dma_start(bb_s[:], bbT_h.ap()[:])
            # weights tiles, free dim padded 25 -> 32 per block for the 32x32
            # DVE stream transposes (padding cols stay uninitialized: they
            # only transpose into wt rows 25..31, which are never read)
            w_s = const.tile([128, 8 * 32], f32)
            for a in range(8):
                nc.sync.dma_start(w_s[:, a * 32:a * 32 + NB], w_h.ap()[a])
            y0_s = const.tile([1, DIM], f32)
            nc.sync.dma_start(y0_s[:], y0_h.ap()[:])
            g_s = const.tile([1, DIM], f32)
            nc.sync.dma_start(g_s[:], g_h.ap()[:])

            # y0-replica output block: stage y0 across 128 SBUF partitions
            # (DMA partition-broadcast needs a DRAM source), then blast it to
            # out[:, 0, :] in 128-row strided writes that read only SBUF.
            rep128 = const.tile([128, DIM], f32)
            nc.sync.dma_start(rep128[:], y0_h.ap().broadcast_to([128, DIM]))
            for j in range((RPC + 127) // 128):
                rows = min(128, RPC - j * 128)
                nc.sync.dma_start(outv[j * 128:j * 128 + rows, 0, :],
                                  rep128[:rows, :])

            # g - y0, broadcast to 25 partitions via a DRAM roundtrip
            gmy0 = const.tile([1, DIM], f32)
            nc.vector.tensor_sub(gmy0[:], g_s[:], y0_s[:])
            gmy0_d = dram.tile([1, DIM], f32)
            nc.sync.dma_start(gmy0_d[:], gmy0[:])
            rep_s = const.tile([NB, DIM], f32)
            nc.sync.dma_start(rep_s[:], gmy0_d[:].broadcast_to([NB, DIM]))

            # w.T via DVE 32x32 stream transposes
            wt_s = const.tile([32, 8 * 128], f32)
            for a in range(8):
                for i in range(4):
                    nc.vector.transpose(
                        wt_s[:, a * 128 + 32 * i:a * 128 + 32 * (i + 1)],
                        w_s[32 * i:32 * (i + 1), a * 32:(a + 1) * 32])

            # rhs rows 0..24: w.T * (g - y0); rows 25/26 (y0, g) via raw DMA
            # (compute-engine APs must start at a quadrant boundary; DMA APs
            # need not)
            rhs_s = const.tile([M, DIM], f32)
            nc.vector.tensor_mul(rhs_s[0:NB, :], wt_s[0:NB, :], rep_s[:])
            nc.gpsimd.dma_start(rhs_s[NB:NB + 1, :], y0_h.ap()[:])
            nc.gpsimd.dma_start(rhs_s[NB + 1:NB + 2, :], g_h.ap()[:])
            # fp32r matmul operands must come from producers that round to
            # fp32r: join-copy both through DVE with float32r outputs
            rhs2 = const.tile([M, DIM], mmdt)
            nc.vector.tensor_copy(rhs2[:], rhs_s[:])
            bb2 = const.tile([M, R2PAD], mmdt)
            nc.vector.tensor_copy(bb2[:], bb_s[:])

            # main matmul: [2502, 27] @ [27, 1024], tiled [128, 512]; each
            # 128-row tile covers 64 t-rows x {y, dy}
            for mt in range(NMT):
                ob = outp.tile([128, DIM], f32)
                for nh in range(2):
                    ps = psMM.tile([128, 512], f32)
                    nc.tensor.matmul(ps[:],
                                     bb2[:, mt * 128:(mt + 1) * 128],
                                     rhs2[:, nh * 512:(nh + 1) * 512],
                                     start=True, stop=True)
                    nc.vector.tensor_copy(ob[:, nh * 512:(nh + 1) * 512], ps[:])
                t0 = mt * 64
                tv = min(64, RPC - t0)
                nc.sync.dma_start(outv[t0:t0 + tv, 1:3, :], ob[:2 * tv, :])

    nc.compile()   # bacc passes: wait legalization (1-wait HW cap), regalloc
    _cache["nc"] = nc
    return nc


def _run(in_maps, **kwargs):
    from concourse.bass_utils import run_bass_kernel_spmd
    return run_bass_kernel_spmd(_program(), in_maps, core_ids=list(range(NCORES)),
                                **kwargs)


def _in_maps(y0, g, weights):
    f32 = np.float32
    y0f = np.ascontiguousarray(np.asarray(y0, f32).reshape(1, DIM))
    gf = np.ascontiguousarray(np.asarray(g, f32).reshape(1, DIM))
    wf = np.ascontiguousarray(np.asarray(weights, f32).reshape(8, 128, NB))
    return [{"bbT": bbT, "y0": y0f, "g": gf, "w": wf}
            for bbT in _basis_slices()]


def kernel(y0, g, weights, **_kwargs):
    res = _run(_in_maps(y0, g, weights))
    outs = [r["out"].reshape(RPC, 3 * DIM) for r in res.results]
    return np.ascontiguousarray(np.concatenate(outs, axis=0)[:T])
